# revision 26
# baseline (speedup 1.0000x reference)
"""Trainium2 Bass kernel for nn_CPSFMemcellFusedReal (scatter_memory).

Contract: kernel(**inputs) takes FULL unsharded numpy inputs (keys as in
reference.setup_inputs()) and returns the FULL [B, S] float32 output.

Strategy (8 NeuronCores, data-parallel over B):
  - shard z / T_star / output rows over the 8 cores (256 rows each)
  - replicate the M-sized store parameters
  - all-reduce the gain.T @ E_eff delta-gradient (mean over B) on-chip

Host-side prep folds every per-m / per-b vector into augmented matmul
operands so the device only runs matmuls + 4 elementwise passes per tile:
  A1'[m,b] = 25 - w_perp[m] * |z_b - z_j[m]|^2   (K=34 matmul)
  A2'[m,b] = proj[b,m]                           (K=34 matmul)
  u = (A2'^2) * (-w_diff[m]) + A1'               (DVE mul + fused STT)
  gain = exp(pi * softplus(u) + ln(alpha_j[m]) - 25*pi)  (2 ACT passes)
Then:
  T_base = gain @ T_hat_eff ;  E = T_base - T_star
  G_part = gain.T @ E  -> AllReduce(sum) over 8 cores -> G
  n = (alpha/B) * ||G||_F ; s = min(CAP/(n+tiny), 1) ; c_g = -s*alpha/B
  out = T_base + c_g * (gain @ G)
"""

import math
import os

import numpy as np

B, M, N, S = 2048, 2048, 32, 256
NCORES = 8
BC = B // NCORES            # 256 rows per core
P = 128
MCH = M // P                # 16 m-chunks
BCH = BC // P               # 2 b-chunks per core
KAUG = N + 2                # 34: [z | znorm | ones] augmented contraction
EPS = 1e-6
MAX_Q = 25.0
CAP = 1.0
PI = float(np.float32(math.pi))

_CACHE: dict = {}


def _patch_act_tables(bacc_mod):
    """Pin all activation instructions to the one table that contains every
    func this kernel uses (exp, ln, square, copy, identity). Without this the
    table-load inserter alternates exp_and_others <-> natural_log per chunk,
    costing ~35 table loads x 1.3us. Stripping the shared funcs from every
    other table forces any correct selector onto natural_log_exp_and_others
    while keeping dict order (act_func_set_id is positional)."""
    if getattr(bacc_mod, "_act_tables_patched", False):
        return
    orig = bacc_mod.get_activation_tables
    keep = "natural_log_exp_and_others"

    def patched(arch):
        t = orig(arch)
        if keep not in t:
            return t
        shared = t[keep]
        return {k: (v if k == keep else (v - shared)) for k, v in t.items()}

    bacc_mod.get_activation_tables = patched
    bacc_mod._act_tables_patched = True


def _build_nc():
    import concourse.mybir as mybir
    import concourse.tile as tile
    from concourse import bacc
    from concourse.masks import make_identity

    _patch_act_tables(bacc)
    fp32 = mybir.dt.float32
    bf16 = mybir.dt.bfloat16
    Alu = mybir.AluOpType
    Act = mybir.ActivationFunctionType

    nc = bacc.Bacc(
        "TRN2",
        target_bir_lowering=False,
        debug=False,
        enable_asserts=False,
        num_devices=NCORES,
    )

    la1 = nc.dram_tensor("la1", [KAUG, M], fp32, kind="ExternalInput").ap()
    la2 = nc.dram_tensor("la2", [KAUG, M], fp32, kind="ExternalInput").ap()
    rhs = nc.dram_tensor("rhs", [KAUG, BC], fp32, kind="ExternalInput").ap()
    nwd = nc.dram_tensor("nwd", [P, MCH], fp32, kind="ExternalInput").ap()
    lnc = nc.dram_tensor("lnc", [P, MCH], fp32, kind="ExternalInput").ap()
    scal = nc.dram_tensor("scal", [1, 2], fp32, kind="ExternalInput").ap()
    th = nc.dram_tensor("th", [M, S], fp32, kind="ExternalInput").ap()
    tst = nc.dram_tensor("tst", [BC, S], fp32, kind="ExternalInput").ap()
    out = nc.dram_tensor("out", [BC, S], fp32, kind="ExternalOutput").ap()

    tiny = float(np.finfo(np.float32).tiny)

    with tile.TileContext(nc) as tc:
        with (
            tc.tile_pool(name="consts", bufs=1) as consts,
            tc.tile_pool(name="persist", bufs=1) as persist,
            tc.tile_pool(name="scratch", bufs=4) as scratch,
            tc.tile_pool(name="dram", bufs=1, space="DRAM") as dram,
        ):
            ident = consts.tile([P, P], bf16)
            make_identity(nc, ident)
            ones_col = consts.tile([P, 1], fp32)
            nc.vector.memset(ones_col, 1.0)
            ones_row = consts.tile([1, P], fp32)
            nc.vector.memset(ones_row, 1.0)

            la1_sb = persist.tile([KAUG, M], fp32)
            la2_sb = persist.tile([KAUG, M], fp32)
            rhs_sb = persist.tile([KAUG, BC], fp32)
            nwd_sb = persist.tile([P, MCH], fp32)
            lnc_sb = persist.tile([P, MCH], fp32)
            scal_sb = persist.tile([1, 2], fp32)
            th_sb = persist.tile([P, MCH * S], fp32)
            tst_sb = persist.tile([P, BCH * S], fp32)
            gain_sb = persist.tile([P, MCH * BC], fp32)
            gainbf_sb = persist.tile([P, MCH * BC], bf16)
            gainT_sb = persist.tile([P, BCH * M], bf16)
            e_sb = persist.tile([P, BCH * S], bf16)
            tb_sb = persist.tile([P, BCH * S], fp32)
            gbf_sb = persist.tile([P, MCH * S], bf16)
            gr_sb = persist.tile([P, MCH * S], bf16)
            nsum_sb = persist.tile([P, MCH], fp32)
            tout_sb = persist.tile([P, BCH * S], fp32)

            # input DMAs; chunked so they spread across DMA queues, triggers
            # alternated between the two HWDGE-capable engines
            trig = [nc.sync, nc.scalar]
            nc.sync.dma_start(rhs_sb, rhs)
            for h in range(4):
                sl = slice(h * M // 4, (h + 1) * M // 4)
                trig[h % 2].dma_start(la1_sb[:, sl], la1[:, sl])
                trig[(h + 1) % 2].dma_start(la2_sb[:, sl], la2[:, sl])
            nc.scalar.dma_start(nwd_sb, nwd)
            nc.sync.dma_start(lnc_sb, lnc)
            nc.scalar.dma_start(scal_sb, scal)
            for i in range(MCH):
                trig[i % 2].dma_start(
                    th_sb[:, i * S:(i + 1) * S], th[i * P:(i + 1) * P, :]
                )
            for bc in range(BCH):
                trig[bc % 2].dma_start(
                    tst_sb[:, bc * S:(bc + 1) * S], tst[bc * P:(bc + 1) * P, :]
                )

            cc_in = dram.tile([M, S], bf16)
            cc_out = dram.tile([M, S], bf16, addr_space="Shared")

            # ---- Phase 1: gain, gain^T, T_base ----
            with (
                tc.tile_pool(name="pa12", bufs=4, space="PSUM") as pa12,
                tc.tile_pool(name="ptb", bufs=2, space="PSUM") as ptb,
                tc.tile_pool(name="ptr", bufs=2, space="PSUM") as ptr,
            ):
                tb_ps = [
                    ptb.tile([P, S], fp32, tag="tb", name=f"tb{b_}")
                    for b_ in range(BCH)
                ]

                for i in range(MCH):
                    a12 = pa12.tile([P, 2 * BC], fp32)
                    a1 = a12[:, 0:BC]
                    a2 = a12[:, BC:2 * BC]
                    lsl = slice(i * P, (i + 1) * P)
                    nc.tensor.matmul(a1, la1_sb[:, lsl], rhs_sb, start=True, stop=True)
                    nc.tensor.matmul(a2, la2_sb[:, lsl], rhs_sb, start=True, stop=True)

                    sq = scratch.tile([P, BC], fp32, tag="sq")
                    nc.scalar.square(sq, a2)
                    u = scratch.tile([P, BC], fp32, tag="u")
                    nc.vector.scalar_tensor_tensor(
                        u, sq, nwd_sb[:, i:i + 1], a1, op0=Alu.mult, op1=Alu.add
                    )
                    # softplus(u) = ln(exp(u) + 1); u <= MAX_Q so exp(u)
                    # never overflows, and exp(u)->0 for very negative u.
                    e1 = scratch.tile([P, BC], fp32, tag="e1")
                    nc.scalar.activation(e1, u, Act.Exp)
                    sp = scratch.tile([P, BC], fp32, tag="sp")
                    nc.scalar.activation(sp, e1, Act.Ln, bias=1.0)
                    gsl = slice(i * BC, (i + 1) * BC)
                    nc.scalar.activation(
                        gain_sb[:, gsl], sp, Act.Exp,
                        scale=PI, bias=lnc_sb[:, i:i + 1],
                    )
                    nc.vector.tensor_copy(gainbf_sb[:, gsl], gain_sb[:, gsl])

                    for bc in range(BCH):
                        gssl = slice(i * BC + bc * P, i * BC + (bc + 1) * P)
                        tr = ptr.tile([P, P], bf16, tag="tr")
                        nc.tensor.transpose(tr, gainbf_sb[:, gssl], ident)
                        nc.vector.tensor_copy(
                            gainT_sb[:, bc * M + i * P: bc * M + (i + 1) * P], tr
                        )
                        nc.tensor.matmul(
                            tb_ps[bc],
                            gain_sb[:, gssl],
                            th_sb[:, i * S:(i + 1) * S],
                            start=(i == 0),
                            stop=(i == MCH - 1),
                        )

                for bc in range(BCH):
                    ssl = slice(bc * S, (bc + 1) * S)
                    nc.vector.tensor_sub(e_sb[:, ssl], tb_ps[bc], tst_sb[:, ssl])
                    nc.scalar.copy(tb_sb[:, ssl], tb_ps[bc])

            # ---- Phase 2: grad partial, all-reduce, delta, final ----
            with (
                tc.tile_pool(name="pgr", bufs=3, space="PSUM") as pgr,
                tc.tile_pool(name="ppp", bufs=2, space="PSUM") as ppp,
                tc.tile_pool(name="psc", bufs=1, space="PSUM") as psc,
            ):
                for i in range(MCH):
                    g = pgr.tile([P, S], fp32, tag="g")
                    for bc in range(BCH):
                        nc.tensor.matmul(
                            g,
                            gainT_sb[:, bc * M + i * P: bc * M + (i + 1) * P],
                            e_sb[:, bc * S:(bc + 1) * S],
                            start=(bc == 0),
                            stop=(bc == BCH - 1),
                        )
                    nc.any.tensor_copy(gr_sb[:, i * S:(i + 1) * S], g)
                    if i % 2 == 1:
                        # ship two m-chunks per DMA; triggers alternate engines
                        dst = cc_in[(i - 1) * P:(i + 1) * P, :].rearrange(
                            "(j p) s -> p j s", p=P
                        )
                        src = gr_sb[:, (i - 1) * S:(i + 1) * S].rearrange(
                            "p (j s) -> p j s", j=2
                        )
                        trig[(i // 2) % 2].dma_start(dst, src)

                nc.gpsimd.collective_compute(
                    "AllReduce",
                    Alu.add,
                    replica_groups=[list(range(NCORES))],
                    ins=[cc_in.opt()],
                    outs=[cc_out.opt()],
                )

                p_ps = [
                    ppp.tile([P, S], fp32, tag="pp", name=f"pp{b_}")
                    for b_ in range(BCH)
                ]
                GG = 4  # G-chunks per DMA / per norm-square instruction
                for q in range(MCH // GG):
                    qsl = slice(q * GG * S, (q + 1) * GG * S)
                    dst = gbf_sb[:, qsl].rearrange("p (j s) -> p j s", j=GG)
                    src = cc_out[q * GG * P:(q + 1) * GG * P, :].rearrange(
                        "(j p) s -> p j s", p=P
                    )
                    trig[q % 2].dma_start(dst, src)
                    sqg = scratch.tile([P, GG * S], fp32, tag="sqg")
                    nc.scalar.activation(
                        sqg, gbf_sb[:, qsl], Act.Square,
                        accum_out=nsum_sb[:, q:q + 1],
                    )
                    for j in range(GG):
                        i = q * GG + j
                        gsl = slice(i * S, (i + 1) * S)
                        for bc in range(BCH):
                            nc.tensor.matmul(
                                p_ps[bc],
                                gainbf_sb[:, i * BC + bc * P: i * BC + (bc + 1) * P],
                                gbf_sb[:, gsl],
                                start=(i == 0),
                                stop=(i == MCH - 1),
                            )

                # n2 = sum over all partitions/chunks of nsum
                nred = scratch.tile([P, 1], fp32, tag="nred")
                nc.vector.tensor_reduce(
                    nred, nsum_sb[:, 0:MCH // 4], axis=mybir.AxisListType.X,
                    op=Alu.add
                )
                n2_ps = psc.tile([1, 1], fp32, tag="n2")
                nc.tensor.matmul(n2_ps, nred, ones_col, start=True, stop=True)

                # n = (alpha/B)*sqrt(n2); s = min(CAP/n, 1) computed sqrt-free
                # as s = exp(0.5*ln(min((CAP*B/alpha)^2 / n2, 1))) using the
                # already-loaded exp/ln table. n2=0 -> 1/n2=inf -> s=1,
                # matching the reference's min(CAP/(0+tiny), 1) = 1.
                rec = scratch.tile([1, 1], fp32, tag="rec")
                nc.vector.reciprocal(rec, n2_ps)
                u2_sb = scratch.tile([1, 1], fp32, tag="u2")
                nc.vector.tensor_scalar(
                    u2_sb, rec, scal_sb[:, 0:1], 1.0, op0=Alu.mult, op1=Alu.min
                )
                l_sb = scratch.tile([1, 1], fp32, tag="l1")
                nc.scalar.activation(l_sb, u2_sb, Act.Ln)
                s_sb = scratch.tile([1, 1], fp32, tag="s1")
                nc.scalar.activation(s_sb, l_sb, Act.Exp, scale=0.5)
                cg_sb = scratch.tile([1, 1], fp32, tag="cg")
                nc.vector.tensor_scalar(
                    cg_sb, s_sb, scal_sb[:, 1:2], None, op0=Alu.mult
                )
                cgb_ps = psc.tile([P, 1], fp32, tag="cgb")
                nc.tensor.matmul(cgb_ps, ones_row, cg_sb, start=True, stop=True)

                for bc in range(BCH):
                    ssl = slice(bc * S, (bc + 1) * S)
                    nc.vector.scalar_tensor_tensor(
                        tout_sb[:, ssl], p_ps[bc], cgb_ps[:, 0:1], tb_sb[:, ssl],
                        op0=Alu.mult, op1=Alu.add,
                    )
                    nc.sync.dma_start(out[bc * P:(bc + 1) * P, :], tout_sb[:, ssl])

    nc.compile()
    return nc


def _host_prep(inputs):
    f32 = np.float32
    z = np.asarray(inputs["z"], f32)
    T_star = np.asarray(inputs["T_star"], f32)
    z_j = np.asarray(inputs["z_j"], f32)
    vec_d_j = np.asarray(inputs["vec_d_j"], f32)
    T_hat_j = np.asarray(inputs["T_hat_j"], f32)
    T_hat_j_delta = np.asarray(inputs["T_hat_j_delta"], f32)
    alpha_j = np.asarray(inputs["alpha_j"], f32)
    sigma_par = np.asarray(inputs["sigma_par"], f32)
    sigma_perp = np.asarray(inputs["sigma_perp"], f32)
    alpha_logit = np.asarray(inputs["alpha_logit"], f32)

    f32eps = np.finfo(np.float32).eps
    sp_par = (np.logaddexp(0.0, sigma_par.astype(np.float64)) + f32eps).astype(f32)
    sp_perp = (np.logaddexp(0.0, sigma_perp.astype(np.float64)) + f32eps).astype(f32)
    w_par = (1.0 / np.maximum(sp_par, f32eps) ** 2).astype(f32)
    w_perp = (1.0 / np.maximum(sp_perp, f32eps) ** 2).astype(f32)
    w_diff = w_par - w_perp

    d_norm = np.linalg.norm(vec_d_j.astype(np.float64), axis=-1, keepdims=True)
    use_proj = d_norm > EPS
    b_dir = np.where(use_proj, vec_d_j / np.maximum(d_norm, 1e-300), 0.0).astype(f32)
    c = np.einsum("mn,mn->m", z_j, b_dir).astype(f32)
    zjn = np.einsum("mn,mn->m", z_j, z_j).astype(f32)
    zn = np.einsum("bn,bn->b", z, z).astype(f32)

    la1 = np.empty((KAUG, M), f32)
    la1[:N] = (2.0 * w_perp[:, None] * z_j).T
    la1[N] = -w_perp
    la1[N + 1] = MAX_Q - w_perp * zjn
    la2 = np.empty((KAUG, M), f32)
    la2[:N] = b_dir.T
    la2[N] = 0.0
    la2[N + 1] = -c

    rhs_full = np.empty((KAUG, B), f32)
    rhs_full[:N] = z.T
    rhs_full[N] = zn
    rhs_full[N + 1] = 1.0

    nwd = np.ascontiguousarray((-w_diff).reshape(MCH, P).T)
    lnc = np.ascontiguousarray(
        (np.log(alpha_j.astype(np.float64)) - math.pi * MAX_Q)
        .astype(f32).reshape(MCH, P).T
    )

    alpha = f32(1.0 / (1.0 + np.exp(-alpha_logit.astype(np.float64))))
    scal = np.array([[(CAP * B / alpha) ** 2, -(alpha / B)]], f32)

    th_eff = T_hat_j + T_hat_j_delta

    return {
        "la1": np.ascontiguousarray(la1),
        "la2": np.ascontiguousarray(la2),
        "rhs_full": np.ascontiguousarray(rhs_full),
        "nwd": nwd,
        "lnc": lnc,
        "scal": scal,
        "th": np.ascontiguousarray(th_eff),
        "tst_full": np.ascontiguousarray(T_star),
    }


def _in_maps(prep):
    maps = []
    for core in range(NCORES):
        bsl = slice(core * BC, (core + 1) * BC)
        maps.append({
            "la1": prep["la1"],
            "la2": prep["la2"],
            "rhs": np.ascontiguousarray(prep["rhs_full"][:, bsl]),
            "nwd": prep["nwd"],
            "lnc": prep["lnc"],
            "scal": prep["scal"],
            "th": prep["th"],
            "tst": np.ascontiguousarray(prep["tst_full"][bsl]),
        })
    return maps


def get_nc():
    if "nc" not in _CACHE:
        _CACHE["nc"] = _build_nc()
    return _CACHE["nc"]


def run_spmd(inputs, **kwargs):
    from concourse.bass_utils import run_bass_kernel_spmd

    nc = get_nc()
    prep = _host_prep(inputs)
    res = run_bass_kernel_spmd(
        nc, _in_maps(prep), core_ids=list(range(NCORES)), **kwargs
    )
    out = np.concatenate(
        [res.results[i]["out"] for i in range(NCORES)], axis=0
    ).astype(np.float32)
    return out, res


def kernel(**inputs):
    out, _ = run_spmd(inputs)
    return out
